# revision 77
# baseline (speedup 1.0000x reference)
"""Cross_Att (spe branch) Trainium2 kernel, v4.

Shapes: B=16, C=256, HW=64x64 -> N=4096 tokens, H=8 heads, d=32, G=32 groups.
Sharding: data-parallel over batch, 2 batches per core on 8 cores.

Algebraic restructuring:
  - gn_w folded into projection weights on the host; device computes only
    rv = rsqrt(var_g + eps) per channel via Newton iteration (var ~= 1 for
    randn inputs), avoiding the ACT Sqrt table (would thrash the Exp table).
  - Post-softmax path is linear:  res = (I + W3) x + b3 with
    W3 = P A^T Wq' diag(rv_x), A = blockdiag(att/Z + bv).  W3T is built on
    the PE (2 small matmuls + transpose) and applied as ONE dense GEMM.
    b3 (tiny, per batch+channel) is exported and added on the host.
  - att accumulates per head ([tok,32] x [tok,33], 33rd col of V = ones
    giving Z "for free").

Engine legality/assignment (GPSIMD can touch neither PSUM nor
TensorScalarPtr, so ACT+DVE carry every PSUM eviction):
  - DVE: bn_stats (the biggest fixed elementwise cost, ~39us), half the
    batch-1 vt evicts, W3 m1b/m2b evicts, attbd chains, res mt1 evicts.
  - ACT: exp, most vt evicts, res mt0 (+ all apply-0-h1) evicts, weight
    scaling via activation(scale=rv).
  - Pool: SBUF-only TensorTensor work - the Newton rsqrt chains against
    constant tiles, partition broadcast, memsets.
  - Output is bf16 (host upcasts and adds b3 + proj_b); batch/phase
    streams are interleaved at 256-token-pair granularity so the serial
    stats->combine->scale and attbd->W3 chains hide under PE work of the
    neighboring phase.
"""

import numpy as np
import ml_dtypes

B, C, N = 16, 256, 4096
H, D = 8, 32
G, GS = 32, 8
EPS = 1e-5
BB = 2            # batches per core
NCORES = 8
KC = 2            # 128-channel chunks
NH = N // 2       # tokens per half
NP = 16           # 256-token pairs per batch

_CACHE = {}


def _build():
    import concourse.bass as bass
    import concourse.bacc as bacc
    import concourse.mybir as mybir
    import concourse.tile as tile

    f32 = mybir.dt.float32
    b16 = mybir.dt.bfloat16
    Alu = mybir.AluOpType
    Act = mybir.ActivationFunctionType

    nc = bacc.Bacc("TRN2", target_bir_lowering=False, debug=False)

    xb_d = nc.dram_tensor("xb", (BB, C, N), b16, kind="ExternalInput")
    yb_d = nc.dram_tensor("yb", (BB, C, N), b16, kind="ExternalInput")
    wkT_d = nc.dram_tensor("wkT", (C, C), b16, kind="ExternalInput")
    wvT_d = nc.dram_tensor("wvT", (C, C), b16, kind="ExternalInput")
    wqT_d = nc.dram_tensor("wqT", (C, C), b16, kind="ExternalInput")
    wqO_d = nc.dram_tensor("wqO", (C, C), b16, kind="ExternalInput")
    pwT_d = nc.dram_tensor("pwT", (C, C), b16, kind="ExternalInput")
    idm_d = nc.dram_tensor("idm", (C, C), b16, kind="ExternalInput")
    bmat_d = nc.dram_tensor("bmat", (128, 128), b16, kind="ExternalInput")
    wqgnb_d = nc.dram_tensor("wqgnb", (C,), f32, kind="ExternalInput")
    wvgnb_d = nc.dram_tensor("wvgnb", (C,), f32, kind="ExternalInput")
    out16_d = nc.dram_tensor("out16", (BB, C, N), b16, kind="ExternalOutput")
    b3_d = nc.dram_tensor("b3o", (BB, C), f32, kind="ExternalOutput")

    with tile.TileContext(nc) as tc:
        import contextlib
        ctx = contextlib.ExitStack()
        with ctx:
            consts = ctx.enter_context(tc.tile_pool(name="consts", bufs=1))
            bigp = ctx.enter_context(tc.tile_pool(name="bigp", bufs=4))
            epool = ctx.enter_context(tc.tile_pool(name="epool", bufs=16))
            wpool = ctx.enter_context(tc.tile_pool(name="wpool", bufs=3))
            stats = ctx.enter_context(tc.tile_pool(name="stats", bufs=3))
            pp = ctx.enter_context(tc.tile_pool(name="pp", bufs=3, space="PSUM"))
            psa = ctx.enter_context(tc.tile_pool(name="psa", bufs=1, space="PSUM"))
            psr = ctx.enter_context(tc.tile_pool(name="psr", bufs=3, space="PSUM"))
            pst = ctx.enter_context(tc.tile_pool(name="pst", bufs=1, space="PSUM"))

            # ---- constants ----
            wkT = consts.tile([128, KC, C], b16)
            wvT = consts.tile([128, KC, C], b16)
            wqT = consts.tile([128, KC, C], b16)
            wqO = consts.tile([128, KC, C], b16)
            pwT = consts.tile([128, KC, C], b16)
            idm = consts.tile([128, KC, C], b16)
            bmat = consts.tile([128, 128], b16)
            wqgnb = consts.tile([128, KC], f32)
            wvgnbr = consts.tile([1, C], f32)
            onesc = consts.tile([128, 1], f32)

            def dma_w2(dst, src_d):
                for kc in range(KC):
                    nc.sync.dma_start(out=dst[:, kc, :],
                                      in_=src_d.ap()[kc * 128:(kc + 1) * 128, :])

            # ---- batch inputs, token halves ----
            xh = [[None, None], [None, None]]
            yh = [[None, None], [None, None]]

            def dma_x(b, h, nq=1):
                hsl = slice(h * NH, (h + 1) * NH)
                t = bigp.tile([128, KC, NH], b16, name=f"xh{b}{h}", tag="xh")
                sap = xb_d.ap()[b].rearrange("(k p) n -> p k n", p=128)[:, :, hsl]
                qn = NH // nq
                for q in range(nq):
                    qsl = slice(q * qn, (q + 1) * qn)
                    nc.sync.dma_start(out=t[:, :, qsl], in_=sap[:, :, qsl])
                xh[b][h] = t

            def dma_y(b, h):
                hsl = slice(h * NH, (h + 1) * NH)
                t = bigp.tile([128, KC, NH], b16, name=f"yh{b}{h}", tag="yh")
                nc.sync.dma_start(
                    out=t, in_=yb_d.ap()[b].rearrange("(k p) n -> p k n", p=128)[:, :, hsl])
                yh[b][h] = t

            dma_x(0, 0, nq=4); dma_x(0, 1, nq=1)
            nc.sync.dma_start(out=bmat, in_=bmat_d.ap())
            dma_w2(wkT, wkT_d)
            dma_w2(wqT, wqT_d)
            nc.sync.dma_start(out=wqgnb, in_=wqgnb_d.ap().rearrange("(k p) -> p k", p=128))
            dma_y(0, 0); dma_y(0, 1)
            dma_w2(wvT, wvT_d)
            dma_w2(wqO, wqO_d)
            dma_w2(pwT, pwT_d)
            dma_w2(idm, idm_d)
            nc.sync.dma_start(out=wvgnbr, in_=wvgnb_d.ap().rearrange("(a n) -> a n", a=1))
            nc.vector.memset(onesc, 1.0)
            epsc = consts.tile([128, KC, 1], f32, name="epsc")
            nhalf = consts.tile([128, KC, 1], f32, name="nhalf")
            c15 = consts.tile([128, KC, 1], f32, name="c15")
            nc.vector.memset(epsc, EPS)
            nc.vector.memset(nhalf, -0.5)
            nc.vector.memset(c15, 1.5)
            actwarm = consts.tile([128, 1], b16, name="actwarm")
            nc.scalar.activation(out=actwarm, in_=onesc, func=Act.Exp)

            # vt ring (4) with persistent ones column (col 32 per head)
            vtt = []
            for s in range(4):
                v = consts.tile([128, 2, H, 33], b16, name=f"vtt{s}")
                nc.vector.memset(v[:, :, :, 32:33], 1.0)
                vtt.append(v)

            dma_x(1, 0); dma_x(1, 1); dma_y(1, 0); dma_y(1, 1)

            NW = NH // 512  # stats windows per half (4)

            def stats_half(src, bn, h):
                for kc in range(KC):
                    for w in range(NW):
                        nc.vector.bn_stats(
                            out=bn[:, kc, h * NW + w, :],
                            in_=src[:, kc, w * 512:(w + 1) * 512])

            def stats_fin(b, nm, bn):
                """aggregate -> per-channel (mean, mean^2+var) bf16 (DVE)"""
                mv = stats.tile([128, KC, 2], f32, name=f"mv{nm}{b}", tag=f"mv{nm}")
                sr = stats.tile([128, KC, 2], b16, name=f"sr{nm}{b}", tag=f"sr{nm}")
                for kc in range(KC):
                    nc.vector.bn_aggr(out=mv[:, kc, :], in_=bn[:, kc, :, :])
                    nc.vector.tensor_copy(out=sr[:, kc, 0:1], in_=mv[:, kc, 0:1])
                    nc.vector.scalar_tensor_tensor(
                        out=sr[:, kc, 1:2], in0=mv[:, kc, 0:1],
                        scalar=mv[:, kc, 0:1], in1=mv[:, kc, 1:2],
                        op0=Alu.mult, op1=Alu.add)
                return sr

            def combine_dve(b, nm, sr):
                """head-critical variant: whole chain on DVE (emitted while
                DVE has no queued bn_stats behind it)."""
                gp = pst.tile([128, KC, 2], f32, name=f"gp{nm}{b}", tag="tiny",
                              bufs=1)
                for kc in range(KC):
                    nc.tensor.matmul(gp[:, kc, :], bmat, sr[:, kc, :],
                                     start=True, stop=True)
                gb = stats.tile([128, KC, 2], f32, name=f"gb{nm}{b}", tag=f"gb{nm}")
                nc.vector.tensor_copy(out=gb, in_=gp)
                m2 = stats.tile([128, KC, 1], f32, name=f"m2{nm}{b}", tag=f"m2{nm}")
                nc.vector.tensor_tensor(out=m2, in0=gb[:, :, 0:1],
                                        in1=gb[:, :, 0:1], op=Alu.mult)
                vx = stats.tile([128, KC, 1], f32, name=f"vx{nm}{b}", tag=f"vx{nm}")
                nc.vector.scalar_tensor_tensor(
                    out=vx, in0=gb[:, :, 1:2], scalar=EPS, in1=m2,
                    op0=Alu.add, op1=Alu.subtract)
                rv = stats.tile([128, KC, 1], f32, name=f"rv{nm}{b}", tag=f"rv{nm}")
                nc.vector.tensor_scalar(out=rv, in0=vx, scalar1=-0.5, scalar2=1.5,
                                        op0=Alu.mult, op1=Alu.add)
                t1 = stats.tile([128, KC, 1], f32, name=f"t1{nm}{b}", tag=f"t1{nm}")
                nc.vector.tensor_tensor(out=t1, in0=rv, in1=rv, op=Alu.mult)
                nc.vector.tensor_tensor(out=t1, in0=vx, in1=t1, op=Alu.mult)
                nc.vector.tensor_scalar(out=t1, in0=t1, scalar1=-0.5, scalar2=1.5,
                                        op0=Alu.mult, op1=Alu.add)
                nc.vector.tensor_tensor(out=rv, in0=rv, in1=t1, op=Alu.mult)
                murv = stats.tile([128, KC, 1], b16, name=f"murv{nm}{b}",
                                  tag=f"murv{nm}")
                nc.vector.tensor_tensor(out=murv, in0=gb[:, :, 0:1], in1=rv,
                                        op=Alu.mult)
                return rv, murv

            def combine(b, nm, sr):
                """group-combine (PE) -> gb evict (DVE tiny) -> Newton rsqrt
                chain on Pool (SBUF only)."""
                gp = pst.tile([128, KC, 2], f32, name=f"gp{nm}{b}", tag="tiny",
                              bufs=1)
                for kc in range(KC):
                    nc.tensor.matmul(gp[:, kc, :], bmat, sr[:, kc, :],
                                     start=True, stop=True)
                gb = stats.tile([128, KC, 2], f32, name=f"gb{nm}{b}", tag=f"gb{nm}")
                nc.vector.tensor_copy(out=gb, in_=gp)
                # Pool supports only TensorTensor of the elementwise family,
                # so the Newton chain uses constant tiles (-0.5, 1.5, eps).
                P = nc.gpsimd
                m2 = stats.tile([128, KC, 1], f32, name=f"m2{nm}{b}", tag=f"m2{nm}")
                P.tensor_tensor(out=m2, in0=gb[:, :, 0:1], in1=gb[:, :, 0:1],
                                op=Alu.mult)
                vx = stats.tile([128, KC, 1], f32, name=f"vx{nm}{b}", tag=f"vx{nm}")
                P.tensor_tensor(out=vx, in0=gb[:, :, 1:2], in1=m2,
                                op=Alu.subtract)
                rv = stats.tile([128, KC, 1], f32, name=f"rv{nm}{b}", tag=f"rv{nm}")
                P.tensor_tensor(out=rv, in0=vx, in1=nhalf, op=Alu.pow)
                murv = stats.tile([128, KC, 1], b16, name=f"murv{nm}{b}",
                                  tag=f"murv{nm}")
                P.tensor_tensor(out=murv, in0=gb[:, :, 0:1], in1=rv, op=Alu.mult)
                return rv, murv

            def scale_x(b, rvx, murvx):
                """wks = wkT*rv_x (Pool, SBUF); bqb = wqgnb - Wq'@murv_x"""
                wks = wpool.tile([128, KC, C], b16, name=f"wks{b}", tag="wks")
                for kc in range(KC):
                    if b == 0:
                        nc.vector.tensor_scalar_mul(out=wks[:, kc, :],
                                                    in0=wkT[:, kc, :],
                                                    scalar1=rvx[:, kc, 0:1])
                    else:
                        nc.scalar.activation(out=wks[:, kc, :],
                                             in_=wkT[:, kc, :],
                                             func=Act.Identity,
                                             scale=rvx[:, kc, 0:1])
                qbp = pst.tile([128, KC], f32, name=f"qbp{b}", tag="tiny", bufs=1)
                for mt in range(KC):
                    for kc in range(KC):
                        nc.tensor.matmul(qbp[:, mt:mt + 1],
                                         wqT[:, kc, mt * 128:(mt + 1) * 128],
                                         murvx[:, kc, :],
                                         start=(kc == 0), stop=(kc == KC - 1))
                bqb = stats.tile([128, KC], b16, name=f"bqb{b}", tag="bqb")
                nc.vector.scalar_tensor_tensor(
                    out=bqb, in0=qbp, scalar=-1.0, in1=wqgnb,
                    op0=Alu.mult, op1=Alu.add)
                return wks, bqb

            def scale_y(b, rvy, murvy):
                """wvs = wvT*rv_y (Pool); bvb = broadcast(wvgnb - Wv'@murv_y)"""
                wvs = wpool.tile([128, KC, C], b16, name=f"wvs{b}", tag="wvs")
                for kc in range(KC):
                    if b == 0:
                        nc.scalar.activation(out=wvs[:, kc, :],
                                             in_=wvT[:, kc, :],
                                             func=Act.Identity,
                                             scale=rvy[:, kc, 0:1])
                    else:
                        nc.vector.tensor_scalar_mul(out=wvs[:, kc, :],
                                                    in0=wvT[:, kc, :],
                                                    scalar1=rvy[:, kc, 0:1])
                bvp = pst.tile([1, C], f32, name=f"bvp{b}", tag="tiny", bufs=1)
                for kc in range(KC):
                    nc.tensor.matmul(bvp, murvy[:, kc, :], wvT[:, kc, :],
                                     start=(kc == 0), stop=(kc == KC - 1))
                bvrow = stats.tile([1, C], f32, name=f"bvrow{b}", tag="bvrow")
                nc.vector.scalar_tensor_tensor(
                    out=bvrow, in0=bvp, scalar=-1.0, in1=wvgnbr,
                    op0=Alu.mult, op1=Alu.add)
                bvb = stats.tile([128, C], f32, name=f"bvb{b}", tag="bvb")
                nc.gpsimd.partition_broadcast(bvb, bvrow)
                return wvs, bvb

            def k1_pairs(b, wks):
                """k1 projections per 256-token pair + exp -> E ring (bf16)."""
                es = [None] * NP

                def emit(p):
                    kp = pp.tile([128, 2, C], f32, name=f"kp{b}{p}", tag="pp")
                    for jj in range(2):
                        c = 2 * p + jj
                        h, tl = c // 16, c % 16
                        for kc in range(KC):
                            nc.tensor.matmul(
                                kp[:, jj, :],
                                xh[b][h][:, kc, tl * 128:(tl + 1) * 128],
                                wks[:, kc, :], start=(kc == 0), stop=(kc == KC - 1))
                    ep = epool.tile([128, 2, C], b16, name=f"ep{b}{p}", tag="et")
                    nc.scalar.activation(out=ep, in_=kp, func=Act.Exp)
                    es[p] = ep
                return es, emit

            def v2att_pairs(b, wvs, es):
                """v2 projections + per-head att accumulation (Z via ones col).
                vt evictions: 3/4 on ACT, 1/4 on DVE."""
                att = psa.tile([128, 2, 4, 33], f32, name=f"att{b}", tag="att")

                def emit_v2(p):
                    vp = pp.tile([128, 2, C], f32, name=f"vp{b}{p}", tag="pp")
                    for jj in range(2):
                        c = 2 * p + jj
                        h, tl = c // 16, c % 16
                        for kc in range(KC):
                            nc.tensor.matmul(
                                vp[:, jj, :],
                                yh[b][h][:, kc, tl * 128:(tl + 1) * 128],
                                wvs[:, kc, :], start=(kc == 0), stop=(kc == KC - 1))
                    src = vp.rearrange("p a (h d) -> p a h d", d=32)
                    dst = vtt[p % 4][:, :, :, 0:32]
                    if b == 1 and p % 2 == 1:
                        nc.vector.tensor_copy(out=dst, in_=src)
                    else:
                        nc.scalar.copy(dst, src)

                def emit_att(p):
                    for jj in range(2):
                        c = 2 * p + jj
                        for t in range(2):
                            for jh in range(4):
                                hd = 128 * t + 32 * jh
                                nc.tensor.matmul(
                                    att[32 * jh:32 * jh + 32, t, jh, :],
                                    es[p][:, jj, hd:hd + 32],
                                    vtt[p % 4][:, jj, 4 * t + jh, :],
                                    start=(c == 0), stop=(c == 2 * NP - 1),
                                    tile_position=(0, 32 * jh))
                return att, emit_v2, emit_att

            def attbd(b, att, bvb):
                """block-diag A = att/Z + bv, bf16 (DVE: reads PSUM)."""
                bds = []
                for t in range(2):
                    bd = stats.tile([128, 128], b16, name=f"bd{b}{t}", tag=f"bd{t}")
                    nc.gpsimd.memset(bd, 0.0)
                    rz = stats.tile([128, 1], f32, name=f"rz{b}{t}", tag=f"rz{t}")
                    for j in range(4):
                        psl = slice(32 * j, 32 * j + 32)
                        nc.vector.reciprocal(out=rz[psl, :],
                                             in_=att[psl, t, j, 32:33])
                    for j in range(4):
                        psl = slice(32 * j, 32 * j + 32)
                        e0 = 128 * t + 32 * j
                        nc.vector.scalar_tensor_tensor(
                            out=bd[psl, 32 * j:32 * j + 32],
                            in0=att[psl, t, j, 0:32], scalar=rz[psl, :],
                            in1=bvb[psl, e0:e0 + 32], op0=Alu.mult, op1=Alu.add)
                    bds.append(bd)
                return bds

            def w3_m1(b, bds):
                m1p = psr.tile([128, KC, C], f32, name=f"m1p{b}", tag="psr")
                for t in range(2):
                    nc.tensor.matmul(m1p[:, t, :], bds[t], wqO[:, t, :],
                                     start=True, stop=True)
                return m1p

            def w3_m2(b, m1p, use_act):
                m1b = wpool.tile([128, KC, C], b16, name=f"m1b{b}", tag="m1b")
                if use_act:
                    nc.scalar.copy(m1b, m1p)
                else:
                    nc.vector.tensor_copy(out=m1b, in_=m1p)
                m2p = psr.tile([128, KC, C], f32, name=f"m2p{b}", tag="psr")
                for mt in range(KC):
                    for et in range(KC):
                        nc.tensor.matmul(m2p[:, mt, :],
                                         pwT[:, et, mt * 128:(mt + 1) * 128],
                                         m1b[:, et, :],
                                         start=(et == 0), stop=(et == KC - 1))
                return m2p

            def w3_tr(b, m2p, use_act):
                m2b = wpool.tile([128, KC, C], b16, name=f"m2b{b}", tag="m2b")
                if use_act:
                    nc.scalar.copy(m2b, m2p)
                else:
                    nc.vector.tensor_copy(out=m2b, in_=m2p)
                w3tp = psr.tile([128, KC, C], b16, name=f"w3tp{b}", tag="psr")
                for mt in range(KC):
                    for ct in range(KC):
                        nc.tensor.matmul(
                            w3tp[:, ct, mt * 128:(mt + 1) * 128],
                            m2b[:, mt, ct * 128:(ct + 1) * 128],
                            idm[:, 0, 0:128], is_transpose=True,
                            start=True, stop=True)
                return w3tp

            def w3_fin(b, w3tp, rvx):
                w3s = wpool.tile([128, KC, C], b16, name=f"w3s{b}", tag="w3s")
                for kc in range(KC):
                    nc.vector.scalar_tensor_tensor(
                        out=w3s[:, kc, :], in0=w3tp[:, kc, :],
                        scalar=rvx[:, kc, 0:1], in1=idm[:, kc, :],
                        op0=Alu.mult, op1=Alu.add)
                return w3s

            def w3_b3(b, bds, bqb):
                """b3 = P @ (A^T bq) -> DRAM (host adds b3 + proj_b)."""
                abqp = pst.tile([128, KC], f32, name=f"abqp{b}", tag="tiny",
                                bufs=1)
                for t in range(2):
                    nc.tensor.matmul(abqp[:, t:t + 1], bds[t], bqb[:, t:t + 1],
                                     start=True, stop=True)
                abqb = stats.tile([128, KC], b16, name=f"abqb{b}", tag="abqb")
                nc.vector.tensor_copy(out=abqb, in_=abqp)
                b3p = pst.tile([128, KC], f32, name=f"b3p{b}", tag="tiny", bufs=1)
                for mt in range(KC):
                    for et in range(KC):
                        nc.tensor.matmul(b3p[:, mt:mt + 1],
                                         pwT[:, et, mt * 128:(mt + 1) * 128],
                                         abqb[:, et:et + 1],
                                         start=(et == 0), stop=(et == KC - 1))
                b3 = stats.tile([128, KC], f32, name=f"b3{b}", tag="b3")
                nc.vector.tensor_copy(out=b3, in_=b3p)
                nc.sync.dma_start(out=b3_d.ap()[b].rearrange("(k p) -> p k", p=128),
                                  in_=b3)

            def apply_chunk(b, w3s, res, h, jl):
                """(I+W3)x chunk -> bf16 evict (ACT mt0, DVE mt1)."""
                nsl = slice(jl * 512, (jl + 1) * 512)
                for mt in range(KC):
                    rp = psr.tile([128, 512], f32, name=f"rp{b}{h}{jl}{mt}",
                                  tag="psr")
                    for kc in range(KC):
                        nc.tensor.matmul(
                            rp, w3s[:, kc, mt * 128:(mt + 1) * 128],
                            xh[b][h][:, kc, nsl],
                            start=(kc == 0), stop=(kc == KC - 1))
                    if mt == 0 or (b == 0 and h == 1):
                        nc.scalar.copy(res[:, mt, nsl], rp)
                    else:
                        nc.vector.tensor_copy(out=res[:, mt, nsl], in_=rp)

            def res_dma(b, res, h, jl=None):
                drp = out16_d.ap()[b].rearrange("(k p) n -> p k n", p=128)
                if jl is None:
                    nc.sync.dma_start(out=drp[:, :, h * NH:(h + 1) * NH],
                                      in_=res)
                else:
                    gsl = slice(h * NH + jl * 512, h * NH + (jl + 1) * 512)
                    nc.sync.dma_start(out=drp[:, :, gsl],
                                      in_=res[:, :, jl * 512:(jl + 1) * 512])

            # ================= emission schedule =================
            bnx = [stats.tile([128, KC, 2 * NW, 6], f32, name=f"bnx{b}", tag="bnx")
                   for b in range(BB)]
            bny = [stats.tile([128, KC, 2 * NW, 6], f32, name=f"bny{b}", tag="bny")
                   for b in range(BB)]

            stats_half(xh[0][0], bnx[0], 0)
            stats_half(xh[0][1], bnx[0], 1)
            srx0 = stats_fin(0, "x", bnx[0])
            rvx0, murvx0 = combine(0, "x", srx0)
            wks0, bqb0 = scale_x(0, rvx0, murvx0)
            stats_half(yh[0][0], bny[0], 0)
            stats_half(yh[0][1], bny[0], 1)
            sry0 = stats_fin(0, "y", bny[0])
            rvy0, murvy0 = combine(0, "y", sry0)

            es0, ek0 = k1_pairs(0, wks0)
            wvs0 = bvb0 = None
            for p in range(NP):
                ek0(p)
                if p == 8:
                    wvs0, bvb0 = scale_y(0, rvy0, murvy0)

            stats_half(xh[1][0], bnx[1], 0)
            stats_half(xh[1][1], bnx[1], 1)
            srx1 = stats_fin(1, "x", bnx[1])
            rvx1, murvx1 = combine(1, "x", srx1)

            att0, ev0, ea0 = v2att_pairs(0, wvs0, es0)
            wks1 = bqb1 = None
            for p in range(NP):
                ev0(p)
                if p >= 2:
                    ea0(p - 2)
                if p == 10:
                    wks1, bqb1 = scale_x(1, rvx1, murvx1)

            stats_half(yh[1][0], bny[1], 0)
            stats_half(yh[1][1], bny[1], 1)
            sry1 = stats_fin(1, "y", bny[1])
            rvy1, murvy1 = combine(1, "y", sry1)

            es1, ek1 = k1_pairs(1, wks1)
            ea0(NP - 2)
            ek1(0)
            ea0(NP - 1)

            # W3-0 build woven between k1-1 pairs
            bds0 = attbd(0, att0, bvb0)
            w3h = {}
            fill = {3: lambda: w3h.__setitem__("m1", w3_m1(0, bds0)),
                    6: lambda: w3h.__setitem__("m2", w3_m2(0, w3h["m1"], False)),
                    9: lambda: w3h.__setitem__("tr", w3_tr(0, w3h["m2"], False)),
                    12: lambda: w3h.__setitem__("w3s", w3_fin(0, w3h["tr"],
                                                              rvx0)),
                    14: lambda: w3_b3(0, bds0, bqb0)}
            wvs1 = bvb1 = None
            for p in range(1, NP):
                if p in fill:
                    fill[p]()
                ek1(p)
                if p == 10:
                    wvs1, bvb1 = scale_y(1, rvy1, murvy1)
            w3s0 = w3h["w3s"]

            # v2att-1 with apply-0 h0 chunks (direct f32 DMA) interleaved
            att1, ev1, ea1 = v2att_pairs(1, wvs1, es1)
            for p in range(NP):
                ev1(p)
                if p >= 2:
                    ea1(p - 2)
                if p % 4 == 3:
                    if p == 3:
                        res00 = bigp.tile([128, KC, NH], b16, name="res00",
                                          tag="res")
                    apply_chunk(0, w3s0, res00, 0, p // 4)
            ea1(NP - 2)
            ea1(NP - 1)
            res_dma(0, res00, 0)

            bds1 = attbd(1, att1, bvb1)
            # W3-1 chain interleaved with apply-0 h1 chunks
            res01 = bigp.tile([128, KC, NH], b16, name="res01", tag="res")
            apply_chunk(0, w3s0, res01, 1, 0)
            res_dma(0, res01, 1, 0)
            m1p1 = w3_m1(1, bds1)
            apply_chunk(0, w3s0, res01, 1, 1)
            res_dma(0, res01, 1, 1)
            m2p1 = w3_m2(1, m1p1, False)
            apply_chunk(0, w3s0, res01, 1, 2)
            res_dma(0, res01, 1, 2)
            w3tp1 = w3_tr(1, m2p1, False)
            apply_chunk(0, w3s0, res01, 1, 3)
            res_dma(0, res01, 1, 3)
            w3s1 = w3_fin(1, w3tp1, rvx1)
            w3_b3(1, bds1, bqb1)

            res10 = bigp.tile([128, KC, NH], b16, name="res10", tag="res")
            for jl in range(4):
                apply_chunk(1, w3s1, res10, 0, jl)
                res_dma(1, res10, 0, jl)
            res11 = bigp.tile([128, KC, NH], b16, name="res11", tag="res")
            for jl in range(4):
                apply_chunk(1, w3s1, res11, 1, jl)
                res_dma(1, res11, 1, jl)

    nc.compile()
    return nc


def _prep_host(x, y, gn_w, gn_b, qkv1_w, qkv2_w, proj_w, proj_b):
    bf16 = ml_dtypes.bfloat16
    f32 = np.float32
    x2 = np.asarray(x, f32).reshape(B, C, N)
    y2 = np.asarray(y, f32).reshape(B, C, N)
    xb = x2.astype(bf16)
    yb = y2.astype(bf16)
    gn_w = np.asarray(gn_w, f32)
    gn_b = np.asarray(gn_b, f32)
    qkv1_w = np.asarray(qkv1_w, f32)
    qkv2_w = np.asarray(qkv2_w, f32)
    proj_w = np.asarray(proj_w, f32)
    Wq = qkv1_w[0:C] * gn_w[None, :]
    Wk = qkv1_w[C:2 * C] * gn_w[None, :]
    Wv = qkv2_w[2 * C:3 * C] * gn_w[None, :]
    wkT = np.ascontiguousarray(Wk.T).astype(bf16)
    wvT = np.ascontiguousarray(Wv.T).astype(bf16)
    wqT = np.ascontiguousarray(Wq.T).astype(bf16)
    wqO = np.ascontiguousarray(Wq).astype(bf16)
    pwT = np.ascontiguousarray(proj_w.T).astype(bf16)
    idm = np.eye(C, dtype=f32).astype(bf16)
    bmat = np.kron(np.eye(16, dtype=f32),
                   np.full((GS, GS), 1.0 / GS, f32)).astype(bf16)
    wqgnb = (qkv1_w[0:C] @ gn_b).astype(f32)
    wvgnb = (qkv2_w[2 * C:3 * C] @ gn_b).astype(f32)
    maps = []
    for core in range(NCORES):
        sl = slice(core * BB, (core + 1) * BB)
        maps.append(dict(
            xb=np.ascontiguousarray(xb[sl]),
            yb=np.ascontiguousarray(yb[sl]),
            wkT=wkT, wvT=wvT, wqT=wqT, wqO=wqO, pwT=pwT, idm=idm, bmat=bmat,
            wqgnb=wqgnb, wvgnb=wvgnb,
        ))
    return maps


def kernel(x, y, gn_w, gn_b, qkv1_w, qkv2_w, proj_w, proj_b, _trace=False):
    from concourse.bass_utils import run_bass_kernel_spmd

    if "nc" not in _CACHE:
        _CACHE["nc"] = _build()
    nc = _CACHE["nc"]
    maps = _prep_host(x, y, gn_w, gn_b, qkv1_w, qkv2_w, proj_w, proj_b)
    res = run_bass_kernel_spmd(nc, maps, core_ids=list(range(NCORES)),
                               trace=_trace)
    pb = np.asarray(proj_b, np.float32)
    outs = []
    for r in res.results:
        full = np.asarray(r["out16"]).astype(np.float32)   # [BB, C, N]
        b3 = np.asarray(r["b3o"])                          # [BB, C]
        full += (b3 + pb[None, :])[:, :, None]
        outs.append(full)
    out = np.concatenate(outs, axis=0).reshape(B, C, 64, 64)
    if _trace:
        return out, res
    return out
